# revision 1
# baseline (speedup 1.0000x reference)
"""DeltaNet model kernel for 8 Trainium2 NeuronCores.

Sharding: data-parallel over batch (2) x tensor-parallel over vocab (4) for
the LM head; each core runs the full 2-layer backbone for its batch element
and computes logits for its 8000-vocab shard.  No inter-core communication.

The delta-rule scan is evaluated in closed "chunked attention" form
(chunk=128): per-chunk inverse of (I + strict_tril(beta * K K^T)) via exact
nilpotent squaring, then all cross-chunk interactions as dense matmuls.

Numerics: float32r (fp32 streamed at bf16 rate, ~12-bit mantissa products,
fp32 accumulate) everywhere except the chunk-inverse iteration (bf16).
All weights are pre-rounded to the f32r grid on the host so DMA-ing them
into f32r tiles is exact.
"""

import sys

for _p in ("/opt/trn_rl_repo",):
    if _p not in sys.path:
        sys.path.insert(0, _p)

import numpy as np

import concourse.bass as bass
import concourse.mybir as mybir
from concourse import bacc
from concourse.bass_utils import run_bass_kernel_spmd
from concourse.tile import TileContext
from concourse.masks import make_identity, make_upper_triangular

P = 128
D = 1024
S = 1024
V = 32000
L = 2
NCH = 8           # token chunks of 128
DSUB = 8          # D / P
VS = V // 4       # vocab shard = 8000
VTS = 63          # padded v-tiles (63*128 = 8064)
VSP = VTS * P

F32 = mybir.dt.float32
F32R = mybir.dt.float32r
BF16 = mybir.dt.bfloat16
I32 = mybir.dt.int32
AF = mybir.ActivationFunctionType
ALU = mybir.AluOpType

EPS_L2 = 1e-6
EPS_RMS = 1e-5
EPS_LN = 1e-5


def ts(i, n):
    return slice(i * n, (i + 1) * n)


def build_program():
    nc = bacc.Bacc("TRN2", target_bir_lowering=False, debug=False, num_devices=8)

    tok_d = nc.dram_tensor("tokens", (P, NCH), I32, kind="ExternalInput").ap()
    emb_d = nc.dram_tensor("emb", (V, D), F32R, kind="ExternalInput").ap()
    wq_d = nc.dram_tensor("wq", (L, P, DSUB, D), F32R, kind="ExternalInput").ap()
    wk_d = nc.dram_tensor("wk", (L, P, DSUB, D), F32R, kind="ExternalInput").ap()
    wv_d = nc.dram_tensor("wv", (L, P, DSUB, D), F32R, kind="ExternalInput").ap()
    wb_d = nc.dram_tensor("wb", (L, P, DSUB, 2), F32R, kind="ExternalInput").ap()
    wo_d = nc.dram_tensor("wo", (L, P, DSUB, D), F32R, kind="ExternalInput").ap()
    lng_d = nc.dram_tensor("lng", (P, DSUB), F32, kind="ExternalInput").ap()
    lnb_d = nc.dram_tensor("lnb", (P, DSUB), F32, kind="ExternalInput").ap()
    hw_d = nc.dram_tensor("hw", (VTS, P, DSUB, P), F32R, kind="ExternalInput").ap()
    out_d = nc.dram_tensor("logits_t", (VSP, S), F32, kind="ExternalOutput").ap()

    with TileContext(nc) as tc:
        _build(nc, tc, tok_d, emb_d, wq_d, wk_d, wv_d, wb_d, wo_d,
               lng_d, lnb_d, hw_d, out_d)
    nc.compile()
    return nc


def _build(nc, tc, tok_d, emb_d, wq_d, wk_d, wv_d, wb_d, wo_d,
           lng_d, lnb_d, hw_d, out_d):
    from contextlib import ExitStack
    ctx = ExitStack()
    pool = ctx.enter_context(tc.tile_pool(name="main", bufs=1))
    ring = ctx.enter_context(tc.tile_pool(name="ring", bufs=2))
    scr = ctx.enter_context(tc.tile_pool(name="scr", bufs=2))
    wpool = ctx.enter_context(tc.tile_pool(name="w", bufs=2))
    hppool = ctx.enter_context(tc.tile_pool(name="hp", bufs=8))
    xpool = ctx.enter_context(tc.tile_pool(name="xs", bufs=7))
    sm2 = ctx.enter_context(tc.tile_pool(name="sm2", bufs=2))
    sm4 = ctx.enter_context(tc.tile_pool(name="sm4", bufs=4))
    sm8 = ctx.enter_context(tc.tile_pool(name="sm8", bufs=8))
    rows = ctx.enter_context(tc.tile_pool(name="rows", bufs=5))
    outp = ctx.enter_context(tc.tile_pool(name="outp", bufs=2))
    hwp = ctx.enter_context(tc.tile_pool(name="hwp", bufs=3))
    dram = ctx.enter_context(tc.tile_pool(name="dram", bufs=1, space="DRAM"))
    pa = ctx.enter_context(tc.tile_pool(name="pa", bufs=4, space="PSUM"))
    pb = ctx.enter_context(tc.tile_pool(name="pb", bufs=4, space="PSUM"))

    # ---- constants ----
    ident_f = pool.tile([P, P], F32, tag="identf")
    make_identity(nc, ident_f[:])
    ident_r = pool.tile([P, P], F32R, tag="identr")
    nc.vector.tensor_copy(ident_r[:], ident_f[:])
    mask_ui = pool.tile([P, P], F32, tag="mui")      # 1 where i <= t (upper incl)
    make_upper_triangular(nc, mask_ui[:], val=1.0, diag=True)
    mask_su = pool.tile([P, P], F32, tag="msu")      # 1 where i < t (strict upper)
    make_upper_triangular(nc, mask_su[:], val=1.0, diag=False)
    ones_f = pool.tile([P, 1], F32, tag="onesf")
    nc.gpsimd.memset(ones_f[:], 1.0)
    ones_r = pool.tile([P, 1], F32R, tag="onesr")    # ones column (f32r)
    nc.vector.tensor_copy(ones_r[:], ones_f[:])
    ones_row = pool.tile([1, P], F32, tag="onesrow")  # ones row for bcast
    nc.gpsimd.memset(ones_row[:], 1.0)
    eps6_t = pool.tile([1, 1], F32, tag="eps6")   # 1e-6 (l2norm)
    nc.gpsimd.memset(eps6_t[:], EPS_L2)
    eps5_t = pool.tile([1, 1], F32, tag="eps5")   # 1e-5 (rms / ln)
    nc.gpsimd.memset(eps5_t[:], EPS_RMS)
    lng_sb = pool.tile([P, DSUB], F32, tag="lng")
    nc.sync.dma_start(lng_sb[:], lng_d[:])
    lnb_sb = pool.tile([P, DSUB], F32, tag="lnb")
    nc.sync.dma_start(lnb_sb[:], lnb_d[:])

    # ---- residual stream (feature-major): xfm[p, do, s] = x[s, do*128+p] ----
    xfm = pool.tile([P, DSUB, S], F32R, tag="xfm")

    # ---- embedding gather + transpose to feature-major ----
    tok_sb = pool.tile([P, NCH], I32, tag="tok")
    nc.sync.dma_start(tok_sb[:], tok_d[:])
    for st in range(NCH):
        xg = ring.tile([P, D], F32R, tag="vc")
        nc.gpsimd.indirect_dma_start(
            out=xg[:], out_offset=None, in_=emb_d[:],
            in_offset=bass.IndirectOffsetOnAxis(ap=tok_sb[:, st:st + 1], axis=0))
        for do in range(DSUB):
            pt = pb.tile([P, 256], F32R, tag="pb")
            nc.tensor.transpose(pt[:, :P], xg[:, ts(do, P)], ident_r[:])
            nc.vector.tensor_copy(xfm[:, do, ts(st, P)], pt[:, :P])

    kfm = pool.tile([P, DSUB, S], F32R, tag="kfm")
    u_tm = pool.tile([P, NCH, D], F32R, tag="u")
    beta_tm = pool.tile([P, NCH], F32, tag="btm")
    beta_fm = pool.tile([1, S], F32, tag="bfm")

    for l in range(L):
        # ==== k projection (feature-major) + silu ====
        for dkt in range(DSUB):  # 128-wide chunks of the dk output dim
            wt = wpool.tile([P, DSUB, P], F32R, tag="w")
            nc.sync.dma_start(wt[:], wk_d[l, :, :, ts(dkt, P)])
            for sh in range(2):       # 512-wide s halves
                ps = pa.tile([P, 512], F32, tag="pa")
                for ko in range(DSUB):
                    nc.tensor.matmul(ps[:], wt[:, ko, :],
                                     xfm[:, ko, ts(sh, 512)],
                                     start=(ko == 0), stop=(ko == DSUB - 1))
                sc = scr.tile([P, 512], F32, tag="scr")
                nc.scalar.activation(sc[:], ps[:], AF.Sigmoid)
                nc.vector.tensor_tensor(kfm[:, dkt, ts(sh, 512)], ps[:], sc[:],
                                        ALU.mult)
        # l2-norm of k rows: sumsq over dk (partition dim) via ones-matmul
        ssk_ps = [pa.tile([P, 512], F32, tag="pa", name=f"ssk{l}_{i}") for i in range(2)]
        for dkt in range(DSUB):
            for sh in range(2):
                sq = scr.tile([P, 512], F32R, tag="scr")
                nc.vector.tensor_tensor(sq[:], kfm[:, dkt, ts(sh, 512)],
                                        kfm[:, dkt, ts(sh, 512)], ALU.mult)
                nc.tensor.matmul(ssk_ps[sh][:1, :], ones_r[:], sq[:],
                                 start=(dkt == 0), stop=(dkt == DSUB - 1))
        rk_row = rows.tile([1, S], F32, tag="rkrow", bufs=1)
        for sh in range(2):
            s_ = rows.tile([1, 512], F32, tag="srow")
            nc.scalar.activation(s_[:], ssk_ps[sh][:1, :], AF.Sqrt, bias=eps6_t[:])
            nc.vector.reciprocal(rk_row[:, ts(sh, 512)], s_[:])
        for sh in range(2):
            psb = pa.tile([P, 512], F32, tag="pa")
            nc.tensor.matmul(psb[:], ones_row[:], rk_row[:, ts(sh, 512)],
                             start=True, stop=True)
            rk_bc = ring.tile([P, 512], F32, tag="bc")
            nc.vector.tensor_copy(rk_bc[:], psb[:])
            for dkt in range(DSUB):
                nc.vector.tensor_tensor(kfm[:, dkt, ts(sh, 512)],
                                        kfm[:, dkt, ts(sh, 512)], rk_bc[:],
                                        ALU.mult)

        # ==== beta (token-major and feature-major) ====
        wbt = pool.tile([P, DSUB, 2], F32R, tag="wb")
        nc.sync.dma_start(wbt[:], wb_d[l])
        for st in range(NCH):
            psb = pb.tile([P, 256], F32, tag="pb")
            for ko in range(DSUB):
                nc.tensor.matmul(psb[:, :2], xfm[:, ko, ts(st, P)], wbt[:, ko, :],
                                 start=(ko == 0), stop=(ko == DSUB - 1))
            nc.scalar.activation(beta_tm[:, st:st + 1], psb[:, :1], AF.Sigmoid)
        for sh in range(2):
            psb = pa.tile([P, 512], F32, tag="pa")
            for ko in range(DSUB):
                nc.tensor.matmul(psb[:2, :], wbt[:, ko, :], xfm[:, ko, ts(sh, 512)],
                                 start=(ko == 0), stop=(ko == DSUB - 1))
            nc.scalar.activation(beta_fm[:, ts(sh, 512)], psb[:1, :], AF.Sigmoid)

        # ==== v = silu(x Wv), token-major, parked in DRAM scratch ====
        v_dram = dram.tile([NCH, P, D], F32R, tag="vdram", name=f"vdram{l}")
        for wc2 in range(4):
            wt = wpool.tile([P, DSUB, 256], F32R, tag="wv", bufs=1,
                            name=f"wv{l}_{wc2}")
            nc.sync.dma_start(wt[:], wv_d[l, :, :, ts(wc2, 256)])
            for st in range(NCH):
                ps = pb.tile([P, 256], F32, tag="pb")
                for ko in range(DSUB):
                    nc.tensor.matmul(ps[:], xfm[:, ko, ts(st, P)], wt[:, ko, :],
                                     start=(ko == 0), stop=(ko == DSUB - 1))
                sc = scr.tile([P, 512], F32, tag="scr")
                nc.scalar.activation(sc[:, :256], ps[:], AF.Sigmoid)
                vstg = ring.tile([P, 256], F32R, tag="vstg")
                nc.vector.tensor_tensor(vstg[:], ps[:], sc[:, :256], ALU.mult)
                nc.sync.dma_start(v_dram[st, :, ts(wc2, 256)], vstg[:])

        # ==== chunk inverses: P_c = diag(beta) T_c^T, T = (I+A)^-1 ====
        Ptiles = []
        for c in range(NCH):
            jps = pb.tile([P, 256], F32, tag="pb")
            for ko in range(DSUB):
                nc.tensor.matmul(jps[:, :P], kfm[:, ko, ts(c, P)],
                                 kfm[:, ko, ts(c, P)],
                                 start=(ko == 0), stop=(ko == DSUB - 1))
            jcc = sm2.tile([P, P], F32, tag="jcc")
            nc.vector.tensor_copy(jcc[:], jps[:, :P])
            # N = strict_tril(beta_row * J);  N^T = strict_triu(beta_col * J)
            tmp = scr.tile([P, 512], F32, tag="scr")
            nc.vector.tensor_scalar_mul(tmp[:, :P], jcc[:], beta_tm[:, c:c + 1])
            tmp2 = scr.tile([P, 512], F32, tag="scr")
            nc.vector.tensor_tensor(tmp2[:, :P], tmp[:, :P], mask_ui[:], ALU.mult)
            n_bf = xpool.tile([P, P], BF16, tag="xs")
            nc.vector.tensor_tensor(n_bf[:], tmp[:, :P], tmp2[:, :P],
                                    ALU.subtract)
            bps = pb.tile([P, 256], F32, tag="pb")
            nc.tensor.matmul(bps[:, :P], ones_row[:], beta_fm[:, ts(c, P)],
                             start=True, stop=True)
            mb = sm2.tile([P, P], F32, tag="mbeta")
            nc.vector.tensor_tensor(mb[:], bps[:, :P], mask_su[:], ALU.mult)
            nt_bf = sm2.tile([P, P], BF16, tag="nt")
            nc.vector.tensor_tensor(nt_bf[:], mb[:], jcc[:], ALU.mult)
            # squarings: X_k = N^(2^k), Xt_k = X_k^T; matmul(lhsT,rhs)=lhsT^T@rhs
            xs = [n_bf]
            xt_prev = nt_bf
            for kk in range(6):
                psx = pb.tile([P, 256], F32, tag="pb")
                nc.tensor.matmul(psx[:, :P], xt_prev[:], xs[-1][:],
                                 start=True, stop=True)
                x_new = xpool.tile([P, P], BF16, tag="xs")
                nc.vector.tensor_copy(x_new[:], psx[:, :P])
                if kk < 5:
                    psxt = pb.tile([P, 256], F32, tag="pb")
                    nc.tensor.matmul(psxt[:, :P], xs[-1][:], xt_prev[:],
                                     start=True, stop=True)
                    xt_new = sm2.tile([P, P], BF16, tag="xt")
                    nc.vector.tensor_copy(xt_new[:], psxt[:, :P])
                    xt_prev = xt_new
                xs.append(x_new)
            # chain: M = I + Y^64; M += Y^(2^k) M (k=5..1); G = M - Y M  (Y=N^T)
            mcur = sm2.tile([P, P], F32, tag="mcur")
            nc.vector.tensor_tensor(mcur[:], ident_f[:], xs[6][:], ALU.add)
            mb16 = sm2.tile([P, P], BF16, tag="mb16")
            nc.vector.tensor_copy(mb16[:], mcur[:])
            for kk in range(5, 0, -1):
                psm = pb.tile([P, 256], F32, tag="pb")
                nc.tensor.matmul(psm[:, :P], xs[kk][:], mb16[:],
                                 start=True, stop=True)
                mnew = sm2.tile([P, P], F32, tag="mcur")
                nc.vector.tensor_tensor(mnew[:], mcur[:], psm[:, :P], ALU.add)
                mcur = mnew
                mb16 = sm2.tile([P, P], BF16, tag="mb16")
                nc.vector.tensor_copy(mb16[:], mcur[:])
            psm = pb.tile([P, 256], F32, tag="pb")
            nc.tensor.matmul(psm[:, :P], xs[0][:], mb16[:], start=True, stop=True)
            gt = sm2.tile([P, P], F32, tag="gt")
            nc.vector.tensor_tensor(gt[:], mcur[:], psm[:, :P], ALU.subtract)
            p_c = sm8.tile([P, P], F32R, tag="pc")
            nc.vector.tensor_scalar_mul(p_c[:], gt[:], beta_tm[:, c:c + 1])
            Ptiles.append(p_c)

        # ==== scan ====
        for cp in range(4):
            c0, c1 = 2 * cp, 2 * cp + 1
            # --- q chunk (256 tokens), silu, feature-major, unnormalized ---
            qfm = ring.tile([P, DSUB, 256], F32R, tag="qfm", bufs=1)
            for dqt in range(DSUB):
                wt = wpool.tile([P, DSUB, P], F32R, tag="w")
                nc.sync.dma_start(wt[:], wq_d[l, :, :, ts(dqt, P)])
                ps = pb.tile([P, 256], F32, tag="pb")
                for ko in range(DSUB):
                    nc.tensor.matmul(ps[:], wt[:, ko, :],
                                     xfm[:, ko, ts(cp, 256)],
                                     start=(ko == 0), stop=(ko == DSUB - 1))
                sc = scr.tile([P, 512], F32, tag="scr")
                nc.scalar.activation(sc[:, :256], ps[:], AF.Sigmoid)
                nc.vector.tensor_tensor(qfm[:, dqt, :], ps[:], sc[:, :256],
                                        ALU.mult)
            # rq for these 256 tokens
            sq_ps = pa.tile([P, 512], F32, tag="pa")
            for dqt in range(DSUB):
                sq = scr.tile([P, 512], F32R, tag="scr")
                nc.vector.tensor_tensor(sq[:, :256], qfm[:, dqt, :],
                                        qfm[:, dqt, :], ALU.mult)
                nc.tensor.matmul(sq_ps[:1, :256], ones_r[:], sq[:, :256],
                                 start=(dqt == 0), stop=(dqt == DSUB - 1))
            s_ = rows.tile([1, 512], F32, tag="srow")
            nc.scalar.activation(s_[:, :256], sq_ps[:1, :256], AF.Sqrt,
                                 bias=eps6_t[:])
            rq_row = rows.tile([1, 512], F32, tag="srow")
            nc.vector.reciprocal(rq_row[:, :256], s_[:, :256])

            for c in (c0, c1):
                # --- v rows for this chunk (from DRAM scratch) ---
                v_c = ring.tile([P, D], F32R, tag="vc")
                nc.sync.dma_start(v_c[:], v_dram[c])
                # --- J pair tiles for j < c (kept across both halves) ---
                jsbs = []
                for jp in range((c + 1) // 2):
                    jps = pb.tile([P, 256], F32, tag="pb")
                    for ko in range(DSUB):
                        nc.tensor.matmul(jps[:], kfm[:, ko, ts(c, P)],
                                         kfm[:, ko, ts(jp, 256)],
                                         start=(ko == 0), stop=(ko == DSUB - 1))
                    jsb = sm4.tile([P, 256], F32R, tag="jsb")
                    nc.vector.tensor_copy(jsb[:], jps[:])
                    jsbs.append(jsb)
                # --- U_c = (T B) V_c - sum_j G_cj U_j ---
                js = list(range(c))
                for half in range(2):
                    gnegs = []
                    for j in js:
                        gps = pb.tile([P, 256], F32, tag="pb")
                        nc.tensor.matmul(gps[:, :P], jsbs[j // 2][:, ts(j % 2, P)],
                                         Ptiles[c][:], start=True, stop=True)
                        gneg = sm8.tile([P, P], F32R, tag="gneg", bufs=3)
                        nc.vector.tensor_scalar_mul(gneg[:], gps[:, :P], -1.0)
                        gnegs.append(gneg)
                    psu = pa.tile([P, 512], F32, tag="pa")
                    nc.tensor.matmul(psu[:], Ptiles[c][:], v_c[:, ts(half, 512)],
                                     start=True, stop=(len(js) == 0))
                    for gi, j in enumerate(js):
                        nc.tensor.matmul(psu[:], gnegs[gi][:],
                                         u_tm[:, j, ts(half, 512)],
                                         start=False, stop=(gi == len(js) - 1))
                    nc.vector.tensor_copy(u_tm[:, c, ts(half, 512)], psu[:])

            # --- H^T pair tiles for this cp ---
            hps = []
            for j in range(c1 + 1):
                php = pb.tile([P, 256], F32, tag="pb")
                for ko in range(DSUB):
                    nc.tensor.matmul(php[:], kfm[:, ko, ts(j, P)], qfm[:, ko, :],
                                     start=(ko == 0), stop=(ko == DSUB - 1))
                hp = hppool.tile([P, 256], F32R, tag="hp")
                if j == c0:
                    nc.vector.tensor_tensor(hp[:, :P], php[:, :P], mask_ui[:],
                                            ALU.mult)
                    nc.vector.tensor_copy(hp[:, P:], php[:, P:])
                elif j == c1:
                    nc.vector.tensor_tensor(hp[:, P:], php[:, P:], mask_ui[:],
                                            ALU.mult)
                else:
                    nc.vector.tensor_copy(hp[:], php[:])
                hps.append(hp)
            # --- O feature-major, accumulate over j per e-tile ---
            on_c = ring.tile([P, DSUB, 256], F32R, tag="on", bufs=1)
            sso_ps = pa.tile([P, 512], F32, tag="pa")
            for wave in range(2):
                opss = []
                for ei in range(4):
                    et = wave * 4 + ei
                    pso = pb.tile([P, 256], F32, tag="pb")
                    for j in range(c1 + 1):
                        if j == c1:
                            nc.tensor.matmul(pso[:, P:], u_tm[:, j, ts(et, P)],
                                             hps[j][:, P:], start=False, stop=True)
                        else:
                            nc.tensor.matmul(pso[:], u_tm[:, j, ts(et, P)],
                                             hps[j][:], start=(j == 0), stop=False)
                    opss.append((et, pso))
                for et, pso in opss:
                    nc.vector.tensor_copy(on_c[:, et, :], pso[:])
                    sq = scr.tile([P, 512], F32R, tag="scr")
                    nc.vector.tensor_tensor(sq[:, :256], on_c[:, et, :],
                                            on_c[:, et, :], ALU.mult)
                    nc.tensor.matmul(sso_ps[:1, :256], ones_r[:], sq[:, :256],
                                     start=(et == 0), stop=(et == DSUB - 1))
            # combined scale row: a = rq / sqrt(rq^2 * sso / D + eps_rms)
            rq2 = rows.tile([1, 512], F32, tag="srow")
            nc.vector.tensor_tensor(rq2[:, :256], rq_row[:, :256],
                                    rq_row[:, :256], ALU.mult)
            nc.vector.tensor_scalar_mul(rq2[:, :256], rq2[:, :256], 1.0 / D)
            ssos = rows.tile([1, 512], F32, tag="srow")
            nc.vector.tensor_tensor(ssos[:, :256], sso_ps[:1, :256], rq2[:, :256],
                                    ALU.mult)
            nc.scalar.activation(ssos[:, :256], ssos[:, :256], AF.Sqrt,
                                 bias=eps5_t[:])
            row_a = rows.tile([1, 512], F32, tag="srow")
            nc.vector.reciprocal(row_a[:, :256], ssos[:, :256])
            nc.vector.tensor_tensor(row_a[:, :256], row_a[:, :256],
                                    rq_row[:, :256], ALU.mult)
            psb = pb.tile([P, 256], F32, tag="pb")
            nc.tensor.matmul(psb[:], ones_row[:], row_a[:, :256],
                             start=True, stop=True)
            a_bc = sm2.tile([P, 256], F32, tag="abc")
            nc.vector.tensor_copy(a_bc[:], psb[:])
            for et in range(DSUB):
                nc.vector.tensor_tensor(on_c[:, et, :], on_c[:, et, :], a_bc[:],
                                        ALU.mult)

            # --- x_next columns for this cp ---
            for do in range(DSUB):
                wt = wpool.tile([P, DSUB, P], F32R, tag="w")
                nc.sync.dma_start(wt[:], wo_d[l, :, :, ts(do, P)])
                psx = pb.tile([P, 256], F32, tag="pb")
                for ko in range(DSUB):
                    nc.tensor.matmul(psx[:], wt[:, ko, :],
                                     on_c[:, ko, :],
                                     start=(ko == 0), stop=(ko == DSUB - 1))
                nc.vector.tensor_copy(xfm[:, do, ts(cp, 256)], psx[:])

    # ==== final layernorm (feature-major) ====
    sum_ps = [pa.tile([P, 512], F32, tag="pa", name=f"lnsum{i}") for i in range(2)]
    ssq_ps = [pa.tile([P, 512], F32, tag="pa", name=f"lnssq{i}") for i in range(2)]
    for do in range(DSUB):
        for sh in range(2):
            nc.tensor.matmul(sum_ps[sh][:1, :], ones_r[:], xfm[:, do, ts(sh, 512)],
                             start=(do == 0), stop=(do == DSUB - 1))
            sq = scr.tile([P, 512], F32R, tag="scr")
            nc.vector.tensor_tensor(sq[:], xfm[:, do, ts(sh, 512)],
                                    xfm[:, do, ts(sh, 512)], ALU.mult)
            nc.tensor.matmul(ssq_ps[sh][:1, :], ones_r[:], sq[:],
                             start=(do == 0), stop=(do == DSUB - 1))
    # per-half: row stats -> broadcast -> apply (xn in place on xfm)
    for sh in range(2):
        mu = rows.tile([1, 512], F32, tag="srow")
        nc.vector.tensor_scalar_mul(mu[:], sum_ps[sh][:1, :], 1.0 / D)
        m2_ = rows.tile([1, 512], F32, tag="srow")
        nc.vector.tensor_scalar_mul(m2_[:], ssq_ps[sh][:1, :], 1.0 / D)
        mu2 = rows.tile([1, 512], F32, tag="srow")
        nc.vector.tensor_tensor(mu2[:], mu[:], mu[:], ALU.mult)
        nc.vector.tensor_tensor(m2_[:], m2_[:], mu2[:], ALU.subtract)
        nc.scalar.activation(mu2[:], m2_[:], AF.Sqrt, bias=eps5_t[:])
        row_a = rows.tile([1, 512], F32, tag="srow")
        nc.vector.reciprocal(row_a[:], mu2[:])
        nc.vector.tensor_scalar_mul(mu[:], mu[:], -1.0)
        row_b = rows.tile([1, 512], F32, tag="srow")
        nc.vector.tensor_tensor(row_b[:], mu[:], row_a[:], ALU.mult)
        psb = pa.tile([P, 512], F32, tag="pa")
        nc.tensor.matmul(psb[:], ones_row[:], row_a[:], start=True, stop=True)
        a_bc = ring.tile([P, 512], F32, tag="bc")
        nc.vector.tensor_copy(a_bc[:], psb[:])
        psb = pa.tile([P, 512], F32, tag="pa")
        nc.tensor.matmul(psb[:], ones_row[:], row_b[:], start=True, stop=True)
        b_bc = ring.tile([P, 512], F32, tag="bc")
        nc.vector.tensor_copy(b_bc[:], psb[:])
        for do in range(DSUB):
            t1 = scr.tile([P, 512], F32, tag="scr")
            nc.vector.tensor_tensor(t1[:], xfm[:, do, ts(sh, 512)], a_bc[:],
                                    ALU.mult)
            nc.vector.tensor_tensor(t1[:], t1[:], b_bc[:], ALU.add)
            nc.vector.tensor_scalar(t1[:], t1[:], lng_sb[:, do:do + 1],
                                    lnb_sb[:, do:do + 1], ALU.mult, ALU.add)
            nc.vector.tensor_copy(xfm[:, do, ts(sh, 512)], t1[:])

    # ==== vocab-shard head: logits_t[vt*128+vv, s] ====
    for vt in range(VTS):
        hwts = []
        for kw in range(2):
            hwt = hwp.tile([P, 4, P], F32R, tag="hw", name=f"hw{vt}_{kw}")
            nc.sync.dma_start(hwt[:], hw_d[vt, :, ts(kw, 4), :])
            hwts.append(hwt)
        for sh in range(2):
            ps = pa.tile([P, 512], F32, tag="pa")
            for ko in range(DSUB):
                nc.tensor.matmul(ps[:], hwts[ko // 4][:, ko % 4, :],
                                 xfm[:, ko, ts(sh, 512)],
                                 start=(ko == 0), stop=(ko == DSUB - 1))
            ot = outp.tile([P, 512], F32, tag="out")
            nc.vector.tensor_copy(ot[:], ps[:])
            nc.sync.dma_start(out_d[ts(vt, P), ts(sh, 512)], ot[:])

    ctx.close()


def _round_f32r(x):
    m, e = np.frexp(x.astype(np.float64))
    return np.ldexp(np.round(m * 4096.0) / 4096.0, e).astype(np.float32)


_CACHE = {}


def _get_program():
    if "nc" not in _CACHE:
        _CACHE["nc"] = build_program()
    return _CACHE["nc"]


def make_in_maps(tokens, emb, Wq, Wk, Wv, Wb, Wo, rms_w, ln_g, ln_b, head_w):
    def arrange_w(w):  # [D, N] -> [128, DSUB, N] with (p, ko) striping of D
        return np.ascontiguousarray(
            _round_f32r(w).reshape(DSUB, P, -1).transpose(1, 0, 2))

    wq_h = np.stack([arrange_w(Wq[l]) for l in range(L)])
    wk_h = np.stack([arrange_w(Wk[l]) for l in range(L)])
    wv_h = np.stack([arrange_w(Wv[l]) for l in range(L)])
    wb_h = np.stack([arrange_w(np.repeat(Wb[l], 2, axis=1)) for l in range(L)])
    wo_h = np.stack([arrange_w(rms_w[l][:, None] * Wo[l]) for l in range(L)])
    emb_h = _round_f32r(emb)
    lng_h = np.ascontiguousarray(ln_g.reshape(DSUB, P).T)
    lnb_h = np.ascontiguousarray(ln_b.reshape(DSUB, P).T)

    in_maps = []
    for core in range(8):
        b, vs = core // 4, core % 4
        hw_pad = np.zeros((D, VSP), np.float32)
        hw_pad[:, :VS] = _round_f32r(head_w[:, ts(vs, VS)])
        hw_h = np.ascontiguousarray(
            hw_pad.reshape(DSUB, P, VTS, P).transpose(2, 1, 0, 3))
        tok_h = np.ascontiguousarray(
            tokens[b].astype(np.int32).reshape(NCH, P).T)
        in_maps.append({
            "tokens": tok_h, "emb": emb_h,
            "wq": wq_h, "wk": wk_h, "wv": wv_h, "wb": wb_h, "wo": wo_h,
            "lng": lng_h, "lnb": lnb_h, "hw": hw_h,
        })
    return in_maps


def assemble_output(results):
    out = np.empty((2, S, V), np.float32)
    for core in range(8):
        b, vs = core // 4, core % 4
        lt = results[core]["logits_t"]          # [VSP, S]
        out[b, :, ts(vs, VS)] = np.ascontiguousarray(lt[:VS]).T
    return out


def kernel(tokens, emb, Wq, Wk, Wv, Wb, Wo, rms_w, ln_g, ln_b, head_w):
    tokens = np.asarray(tokens)
    args = [np.asarray(a, np.float32) for a in
            (emb, Wq, Wk, Wv, Wb, Wo, rms_w, ln_g, ln_b, head_w)]
    nc = _get_program()
    in_maps = make_in_maps(tokens, *args)
    res = run_bass_kernel_spmd(nc, in_maps, core_ids=list(range(8)),
                               trace=bool(_CACHE.get("trace")))
    _CACHE["last_result"] = res
    return assemble_output(res.results)



# revision 12
# speedup vs baseline: 1.6945x; 1.6945x over previous
"""DeltaNet model kernel for 8 Trainium2 NeuronCores.

Sharding: data-parallel over batch (2) x tensor-parallel over vocab (4) for
the LM head; each core runs the full 2-layer backbone for its batch element
and computes logits for its 8000-vocab shard.  No inter-core communication.

The delta-rule scan is evaluated in closed "chunked attention" form
(chunk=128): per-chunk inverse of (I + strict_tril(beta * K K^T)) via exact
nilpotent squaring (dual B/B^T ladders, product-form chain evaluated with
identity-matmul accumulation in PSUM), then all cross-chunk interactions as
dense matmuls.

v2: weights resident in SBUF (2MB half-slabs, double buffered), k/q/v in
bf16 via scalar-engine Silu directly from PSUM, v kept in SBUF (no DRAM
roundtrip), level-parallel inverse across chunks, hoisted G tiles,
fast approximate reciprocal for row stats, f32r broadcast rows, deep
head-weight prefetch.
"""

import sys

for _p in ("/opt/trn_rl_repo",):
    if _p not in sys.path:
        sys.path.insert(0, _p)

import ml_dtypes
import numpy as np

import concourse.bass as bass
import concourse.mybir as mybir
from concourse import bacc
from concourse.bass_utils import run_bass_kernel_spmd
from concourse.tile import TileContext
from concourse.masks import (
    make_identity,
    make_upper_triangular,
    make_lower_triangular,
)

P = 128
D = 1024
S = 1024
V = 32000
L = 2
NCH = 8           # token chunks of 128
DSUB = 8          # D / P
VS = V // 4       # vocab shard = 8000
VTS = 63          # padded v-tiles (63*128 = 8064)
VSP = VTS * P

F32 = mybir.dt.float32
F32R = mybir.dt.float32r
BF16 = mybir.dt.bfloat16
I32 = mybir.dt.int32
AF = mybir.ActivationFunctionType
ALU = mybir.AluOpType

EPS_L2 = 1e-6
EPS_RMS = 1e-5
EPS_LN = 1e-5


def ts(i, n):
    return slice(i * n, (i + 1) * n)


def build_program():
    nc = bacc.Bacc("TRN2", target_bir_lowering=False, debug=False, num_devices=8)

    tok_d = nc.dram_tensor("tokens", (P, NCH), I32, kind="ExternalInput").ap()
    emb_d = nc.dram_tensor("emb", (V, D), F32R, kind="ExternalInput").ap()
    wq_d = nc.dram_tensor("wq", (L, P, DSUB, D), F32R, kind="ExternalInput").ap()
    wk_d = nc.dram_tensor("wk", (L, P, DSUB, D), F32R, kind="ExternalInput").ap()
    wv_d = nc.dram_tensor("wv", (L, P, DSUB, D), F32R, kind="ExternalInput").ap()
    wb_d = nc.dram_tensor("wb", (L, P, DSUB, 2), F32R, kind="ExternalInput").ap()
    wo_d = nc.dram_tensor("wo", (L, P, DSUB, D), BF16, kind="ExternalInput").ap()
    lng_d = nc.dram_tensor("lng", (P, DSUB), F32, kind="ExternalInput").ap()
    lnb_d = nc.dram_tensor("lnb", (P, DSUB), F32, kind="ExternalInput").ap()
    hw_d = nc.dram_tensor("hw", (VTS, P, DSUB, P), F32R, kind="ExternalInput").ap()
    out_d = nc.dram_tensor("logits_t", (VSP, S), F32, kind="ExternalOutput").ap()

    with TileContext(nc) as tc:
        _build(nc, tc, tok_d, emb_d, wq_d, wk_d, wv_d, wb_d, wo_d,
               lng_d, lnb_d, hw_d, out_d)
    nc.compile()
    return nc


def _build(nc, tc, tok_d, emb_d, wq_d, wk_d, wv_d, wb_d, wo_d,
           lng_d, lnb_d, hw_d, out_d):
    from contextlib import ExitStack
    ctx = ExitStack()

    def ecopy(i, out, in_):
        if i % 2 == 0:
            nc.vector.tensor_copy(out, in_)
        else:
            nc.scalar.copy(out, in_)
    pool = ctx.enter_context(tc.tile_pool(name="main", bufs=1))
    wpool = ctx.enter_context(tc.tile_pool(name="w", bufs=2))
    ring = ctx.enter_context(tc.tile_pool(name="ring", bufs=2))
    scr = ctx.enter_context(tc.tile_pool(name="scr", bufs=2))
    bp = ctx.enter_context(tc.tile_pool(name="bp", bufs=2))
    sqp = ctx.enter_context(tc.tile_pool(name="sqp", bufs=2))
    invp = ctx.enter_context(tc.tile_pool(name="invp", bufs=54))
    rpool = ctx.enter_context(tc.tile_pool(name="rp", bufs=12))
    ppool = ctx.enter_context(tc.tile_pool(name="pp", bufs=8))
    jpool = ctx.enter_context(tc.tile_pool(name="jp", bufs=8))
    gpool = ctx.enter_context(tc.tile_pool(name="gp", bufs=14))
    hpool = ctx.enter_context(tc.tile_pool(name="hp", bufs=9))
    onp = ctx.enter_context(tc.tile_pool(name="onp", bufs=2))
    rows = ctx.enter_context(tc.tile_pool(name="rows", bufs=3))
    hwp = ctx.enter_context(tc.tile_pool(name="hwp", bufs=2))
    outp = ctx.enter_context(tc.tile_pool(name="outp", bufs=2))
    pa = ctx.enter_context(tc.tile_pool(name="pa", bufs=2, space="PSUM"))
    pb = ctx.enter_context(tc.tile_pool(name="pb", bufs=3, space="PSUM"))
    ptr = ctx.enter_context(tc.tile_pool(name="ptr", bufs=1, space="PSUM"))
    prow = ctx.enter_context(tc.tile_pool(name="prow", bufs=2, space="PSUM"))

    # ---- constants ----
    ident_f = pool.tile([P, P], F32, tag="identf")
    make_identity(nc, ident_f[:])
    ident_r = pool.tile([P, P], F32R, tag="identr")
    nc.vector.tensor_copy(ident_r[:], ident_f[:])
    ident_b = pool.tile([P, P], BF16, tag="identb")
    nc.vector.tensor_copy(ident_b[:], ident_f[:])
    mask_ui = pool.tile([P, P], F32, tag="mui")      # 1 where i <= t (upper incl)
    make_upper_triangular(nc, mask_ui[:], val=1.0, diag=True)
    mask_sl = pool.tile([P, P], F32, tag="msl")      # 1 where i > t (strict lower)
    make_lower_triangular(nc, mask_sl[:], val=1.0, diag=False)
    ones_col_f = pool.tile([P, 1], F32, tag="onescf")
    nc.gpsimd.memset(ones_col_f[:], 1.0)
    invd_col_f = pool.tile([P, 1], F32, tag="invdcf")
    nc.gpsimd.memset(invd_col_f[:], 1.0 / D)
    ones_col_b = pool.tile([P, 1], BF16, tag="onescb")
    nc.vector.tensor_copy(ones_col_b[:], ones_col_f[:])
    invd_col_b = pool.tile([P, 1], BF16, tag="invdcb")
    nc.vector.tensor_copy(invd_col_b[:], invd_col_f[:])
    ones_row_f = pool.tile([1, P], F32, tag="onesrf")
    nc.gpsimd.memset(ones_row_f[:], 1.0)
    ones_col_r = pool.tile([P, 1], F32R, tag="onescr")
    nc.vector.tensor_copy(ones_col_r[:], ones_col_f[:])
    eps6_t = pool.tile([1, 1], F32, tag="eps6")   # 1e-6 (l2norm)
    nc.gpsimd.memset(eps6_t[:], EPS_L2)
    eps5_t = pool.tile([1, 1], F32, tag="eps5")   # 1e-5 (rms / ln)
    nc.gpsimd.memset(eps5_t[:], EPS_RMS)
    lng_sb = pool.tile([P, DSUB], F32, tag="lng")
    nc.sync.dma_start(lng_sb[:], lng_d[:])
    lnb_sb = pool.tile([P, DSUB], F32, tag="lnb")
    nc.sync.dma_start(lnb_sb[:], lnb_d[:])

    # ---- persistent activations ----
    # xfm[p, do, s] = x[s, do*128+p]  (feature-major residual)
    xfm = pool.tile([P, DSUB, S], F32R, tag="xfm")
    kfm = pool.tile([P, DSUB, S], BF16, tag="kfm")
    qfm = pool.tile([P, DSUB, S], BF16, tag="qfm")
    # v token-major: vtm[p, c, e] = v[c*128+p, e]
    vtm = pool.tile([P, NCH, D], BF16, tag="vtm")
    # u token-major: utm[p, c, e]
    utm = pool.tile([P, NCH, D], BF16, tag="utm")
    beta_tm = pool.tile([P, NCH], F32, tag="btm")
    negb_tm = pool.tile([P, NCH], F32, tag="nbtm")
    rq_row = pool.tile([1, S], F32, tag="rqrow")

    # ---- embedding gather + transpose to feature-major ----
    tok_sb = pool.tile([P, NCH], I32, tag="tok")
    nc.sync.dma_start(tok_sb[:], tok_d[:])
    for st in range(NCH):
        xg = ring.tile([P, D], F32R, tag="xg")
        nc.gpsimd.indirect_dma_start(
            out=xg[:], out_offset=None, in_=emb_d[:],
            in_offset=bass.IndirectOffsetOnAxis(ap=tok_sb[:, st:st + 1], axis=0))
        for dp in range(4):      # two transposes per PSUM tile
            pt = pb.tile([P, 256], F32R, tag="pbt")
            nc.tensor.transpose(pt[:, :P], xg[:, ts(2 * dp, P)], ident_r[:])
            nc.tensor.transpose(pt[:, P:], xg[:, ts(2 * dp + 1, P)], ident_r[:])
            ecopy(dp, xfm[:, 2 * dp:2 * dp + 2, ts(st, P)], pt[:])

    for l in range(L):
        # ==== k / q projections + silu (bf16, unnormalized) ====
        for dst, w_d in ((kfm, wk_d), (qfm, wq_d)):
            for wq4 in range(4):
                wt = wpool.tile([P, DSUB, 256], F32R, tag="w")
                nc.sync.dma_start(wt[:], w_d[l, :, :, ts(wq4, 256)])
                for dl in range(2):
                    dkt = wq4 * 2 + dl
                    for sh in range(2):
                        ps = pa.tile([P, 512], F32, tag="pa")
                        for ko in range(DSUB):
                            nc.tensor.matmul(ps[:], wt[:, ko, ts(dl, P)],
                                             xfm[:, ko, ts(sh, 512)],
                                             start=(ko == 0), stop=(ko == DSUB - 1))
                        nc.scalar.activation(dst[:, dkt, ts(sh, 512)], ps[:],
                                             AF.Silu)

        # ==== l2-norm row stats (k then q), sqrt grouped ====
        rk_bcs = []
        for src in (kfm, qfm):
            ss_ps = [prow.tile([P, 512], F32, tag="prow",
                               name=f"ss{l}_{0 if src is kfm else 1}_{i}")
                     for i in range(2)]
            for dkt in range(DSUB):
                for sh in range(2):
                    sq = sqp.tile([P, 512], BF16, tag="sq")
                    nc.scalar.activation(sq[:], src[:, dkt, ts(sh, 512)],
                                         AF.Square)
                    nc.tensor.matmul(ss_ps[sh][:1, :], ones_col_b[:], sq[:],
                                     start=(dkt == 0), stop=(dkt == DSUB - 1))
            for sh in range(2):
                s_ = rows.tile([1, 512], F32, tag="srow")
                nc.scalar.activation(s_[:], ss_ps[sh][:1, :], AF.Sqrt,
                                     bias=eps6_t[:])
                if src is kfm:
                    rrow = rows.tile([1, 512], F32, tag="srow")
                    nc.vector.reciprocal_approx_fast(out=rrow[:], in_=s_[:])
                    psb = pb.tile([P, 256], F32, tag="pbt", name=f"rkb{l}{sh}a")
                    psb2 = pb.tile([P, 256], F32, tag="pbt", name=f"rkb{l}{sh}b")
                    nc.tensor.matmul(psb[:], ones_row_f[:], rrow[:, :256],
                                     start=True, stop=True)
                    nc.tensor.matmul(psb2[:], ones_row_f[:], rrow[:, 256:],
                                     start=True, stop=True)
                    rk_bc = ring.tile([P, 512], BF16, tag="rkbc", bufs=2)
                    nc.vector.tensor_copy(rk_bc[:, :256], psb[:])
                    nc.vector.tensor_copy(rk_bc[:, 256:], psb2[:])
                    rk_bcs.append(rk_bc)
                else:
                    nc.vector.reciprocal_approx_fast(
                        out=rq_row[:, ts(sh, 512)], in_=s_[:])
        # normalize k in place (bf16)
        for dkt in range(DSUB):
            for sh in range(2):
                nc.vector.tensor_tensor(kfm[:, dkt, ts(sh, 512)],
                                        kfm[:, dkt, ts(sh, 512)],
                                        rk_bcs[sh][:], ALU.mult)

        # ==== v = silu(x Wv), token-major bf16 in SBUF ====
        for wq4 in range(4):
            wt = wpool.tile([P, DSUB, 256], F32R, tag="w")
            nc.sync.dma_start(wt[:], wv_d[l, :, :, ts(wq4, 256)])
            col = wq4 * 256
            for st in range(NCH):
                ps = pb.tile([P, 256], F32, tag="pbt")
                for ko in range(DSUB):
                    nc.tensor.matmul(ps[:], xfm[:, ko, ts(st, P)],
                                     wt[:, ko, :],
                                     start=(ko == 0), stop=(ko == DSUB - 1))
                nc.scalar.activation(vtm[:, st, col:col + 256], ps[:], AF.Silu)

        # ==== beta (token-major) ====
        wbt = pool.tile([P, DSUB, 2], F32R, tag="wb")
        nc.sync.dma_start(wbt[:], wb_d[l])
        for st in range(NCH):
            psb = pb.tile([P, 256], F32, tag="pbt")
            for ko in range(DSUB):
                nc.tensor.matmul(psb[:, :2], xfm[:, ko, ts(st, P)], wbt[:, ko, :],
                                 start=(ko == 0), stop=(ko == DSUB - 1))
            nc.scalar.activation(beta_tm[:, st:st + 1], psb[:, :1], AF.Sigmoid)
        nc.vector.tensor_scalar_mul(negb_tm[:], beta_tm[:], -1.0)

        # ==== chunk inverses: P_c = diag(beta) T_c^T ====
        # T = (I+A)^-1, A = strict_tril(beta*J).  B = -A, C_k = (B^(2^k))^T.
        # T^T = prod_k (I + C_k); evaluated as residual R (M = I + R).
        Ptiles = [None] * NCH
        for wave in range(2):
            cs = [wave * 4 + i for i in range(4)]
            bt = {}
            ct = {}
            for c in cs:
                jps = pb.tile([P, 256], F32, tag="pbt")
                for ko in range(DSUB):
                    nc.tensor.matmul(jps[:, :P], kfm[:, ko, ts(c, P)],
                                     kfm[:, ko, ts(c, P)],
                                     start=(ko == 0), stop=(ko == DSUB - 1))
                t1 = bp.tile([P, P], F32, tag="bp")
                nc.vector.tensor_scalar_mul(t1[:], jps[:, :P],
                                            negb_tm[:, c:c + 1])
                b0 = invp.tile([P, P], BF16, tag="inv")
                nc.vector.tensor_tensor(b0[:], t1[:], mask_sl[:], ALU.mult)
                bt[(0, c)] = b0
            for c in cs:
                ptp = ptr.tile([P, 256], BF16, tag="pbb")
                nc.tensor.transpose(ptp[:, :P], bt[(0, c)][:], ident_b[:])
                c0 = invp.tile([P, P], BF16, tag="inv")
                ecopy(c, c0[:], ptp[:, :P])
                ct[(0, c)] = c0
            for kk in range(6):
                for c in cs:
                    psx = pb.tile([P, 256], F32, tag="pbt")
                    nc.tensor.matmul(psx[:, :P], bt[(kk, c)][:], ct[(kk, c)][:],
                                     start=True, stop=True)
                    cn = invp.tile([P, P], BF16, tag="inv")
                    ecopy(c, cn[:], psx[:, :P])
                    ct[(kk + 1, c)] = cn
                if kk < 5:
                    for c in cs:
                        psx = pb.tile([P, 256], F32, tag="pbt")
                        nc.tensor.matmul(psx[:, :P], ct[(kk, c)][:],
                                         bt[(kk, c)][:], start=True, stop=True)
                        bn = invp.tile([P, P], BF16, tag="inv")
                        ecopy(c, bn[:], psx[:, :P])
                        bt[(kk + 1, c)] = bn
            # chain: R_6 = C_6; R_k = C_k + R + C_k R;  M = I + R_0
            rt = {c: ct[(6, c)] for c in cs}
            for kk in range(5, -1, -1):
                for c in cs:
                    psm = pb.tile([P, 256], F32, tag="pbt")
                    nc.tensor.matmul(psm[:, :P], ident_b[:], ct[(kk, c)][:],
                                     start=True, stop=False)
                    nc.tensor.matmul(psm[:, :P], ident_b[:], rt[c][:],
                                     start=False, stop=(kk != 0))
                    nc.tensor.matmul(psm[:, :P], bt[(kk, c)][:], rt[c][:],
                                     start=False, stop=(kk != 0))
                    if kk == 0:
                        nc.tensor.matmul(psm[:, :P], ident_b[:], ident_b[:],
                                         start=False, stop=True)
                        p_c = ppool.tile([P, P], BF16, tag="pc")
                        nc.vector.tensor_scalar_mul(p_c[:], psm[:, :P],
                                                    beta_tm[:, c:c + 1])
                        Ptiles[c] = p_c
                    else:
                        rn = rpool.tile([P, P], BF16, tag="rt")
                        ecopy(c, rn[:], psm[:, :P])
                        rt[c] = rn

        # ==== scan ====
        for cp in range(4):
            c0, c1 = 2 * cp, 2 * cp + 1
            # --- J pair tiles (j < c) ---
            jsbs = {}
            for c in (c0, c1):
                for jp in range((c + 1) // 2):
                    jps = pb.tile([P, 256], F32, tag="pbt")
                    for ko in range(DSUB):
                        nc.tensor.matmul(jps[:], kfm[:, ko, ts(c, P)],
                                         kfm[:, ko, ts(jp, 256)],
                                         start=(ko == 0), stop=(ko == DSUB - 1))
                    jsb = jpool.tile([P, 256], BF16, tag="jsb")
                    ecopy(jp, jsb[:], jps[:])
                    jsbs[(c, jp)] = jsb
            # --- G tiles (hoisted, computed once per (c, j)) ---
            gnegs = {}
            for c in (c0, c1):
                for j in range(c):
                    gps = pb.tile([P, 256], F32, tag="pbt")
                    nc.tensor.matmul(gps[:, :P],
                                     jsbs[(c, j // 2)][:, ts(j % 2, P)],
                                     Ptiles[c][:], start=True, stop=True)
                    gneg = gpool.tile([P, P], BF16, tag="gneg")
                    if j % 2 == 0:
                        nc.vector.tensor_scalar_mul(gneg[:], gps[:, :P], -1.0)
                    else:
                        nc.scalar.mul(gneg[:], gps[:, :P], -1.0)
                    gnegs[(c, j)] = gneg
            # --- H^T pair tiles ---
            hps = []
            for j in range(c1 + 1):
                php = pb.tile([P, 256], F32, tag="pbt")
                for ko in range(DSUB):
                    nc.tensor.matmul(php[:], kfm[:, ko, ts(j, P)],
                                     qfm[:, ko, ts(cp, 256)],
                                     start=(ko == 0), stop=(ko == DSUB - 1))
                hp = hpool.tile([P, 256], BF16, tag="hp")
                if j == c0:
                    nc.vector.tensor_tensor(hp[:, :P], php[:, :P], mask_ui[:],
                                            ALU.mult)
                    nc.scalar.copy(hp[:, P:], php[:, P:])
                elif j == c1:
                    nc.scalar.copy(hp[:, :P], php[:, :P])
                    nc.vector.tensor_tensor(hp[:, P:], php[:, P:], mask_ui[:],
                                            ALU.mult)
                else:
                    ecopy(j, hp[:], php[:])
                hps.append(hp)
            # --- U_c = (T B) V_c - sum_j G_cj U_j ---
            for c in (c0, c1):
                for half in range(2):
                    psu = pa.tile([P, 512], F32, tag="pa")
                    nc.tensor.matmul(psu[:], Ptiles[c][:],
                                     vtm[:, c, ts(half, 512)],
                                     start=True, stop=(c == 0))
                    for j in range(c):
                        nc.tensor.matmul(psu[:], gnegs[(c, j)][:],
                                         utm[:, j, ts(half, 512)],
                                         start=False, stop=(j == c - 1))
                    ecopy(half, utm[:, c, ts(half, 512)], psu[:])
            # --- O feature-major (bf16) + sumsq ---
            on_c = onp.tile([P, DSUB, 256], BF16, tag="on")
            sso_ps = prow.tile([P, 512], F32, tag="prow")
            for et in range(DSUB):
                pso = pb.tile([P, 256], F32, tag="pbt")
                for j in range(c1 + 1):
                    if j == c1:
                        nc.tensor.matmul(pso[:, P:], utm[:, j, ts(et, P)],
                                         hps[j][:, P:], start=False, stop=True)
                    else:
                        nc.tensor.matmul(pso[:], utm[:, j, ts(et, P)],
                                         hps[j][:], start=(j == 0), stop=False)
                ecopy(et, on_c[:, et, :], pso[:])
                sq = sqp.tile([P, 512], BF16, tag="sq")
                nc.scalar.activation(sq[:, :256], on_c[:, et, :], AF.Square)
                nc.tensor.matmul(sso_ps[:1, :256], invd_col_b[:], sq[:, :256],
                                 start=(et == 0), stop=(et == DSUB - 1))
            # combined scale row: a = rq / sqrt(rq^2 * sso / D + eps_rms)
            rq2 = rows.tile([1, 512], F32, tag="srow")
            nc.vector.tensor_tensor(rq2[:, :256], rq_row[:, ts(cp, 256)],
                                    rq_row[:, ts(cp, 256)], ALU.mult)
            ssos = rows.tile([1, 512], F32, tag="srow")
            nc.vector.tensor_tensor(ssos[:, :256], sso_ps[:1, :256],
                                    rq2[:, :256], ALU.mult)
            nc.scalar.activation(ssos[:, :256], ssos[:, :256], AF.Sqrt,
                                 bias=eps5_t[:])
            row_a = rows.tile([1, 512], F32, tag="srow")
            nc.vector.reciprocal_approx_fast(out=row_a[:, :256],
                                             in_=ssos[:, :256])
            nc.vector.tensor_tensor(row_a[:, :256], row_a[:, :256],
                                    rq_row[:, ts(cp, 256)], ALU.mult)
            psb = pb.tile([P, 256], F32, tag="pbt")
            nc.tensor.matmul(psb[:], ones_row_f[:], row_a[:, :256],
                             start=True, stop=True)
            a_bc = ring.tile([P, 256], BF16, tag="abc")
            nc.vector.tensor_copy(a_bc[:], psb[:])
            for et in range(DSUB):
                nc.vector.tensor_tensor(qfm[:, et, ts(cp, 256)], on_c[:, et, :],
                                        a_bc[:], ALU.mult)


        # ==== x_next = o @ Wo (o parked in qfm storage) ====
        for wq4 in range(4):
            wt = wpool.tile([P, DSUB, 256], BF16, tag="wo", bufs=2)
            nc.sync.dma_start(wt[:], wo_d[l, :, :, ts(wq4, 256)])
            for dl in range(2):
                do = wq4 * 2 + dl
                for sh in range(2):
                    psx = pa.tile([P, 512], F32, tag="pa")
                    for ko in range(DSUB):
                        nc.tensor.matmul(psx[:], wt[:, ko, ts(dl, P)],
                                         qfm[:, ko, ts(sh, 512)],
                                         start=(ko == 0), stop=(ko == DSUB - 1))
                    ecopy(do, xfm[:, do, ts(sh, 512)], psx[:])

    # ==== head weight prefetch (first tiles) ====
    hwts = {}
    for vt in range(2):
        hwt = hwp.tile([P, DSUB, P], F32R, tag="hw")
        nc.sync.dma_start(hwt[:], hw_d[vt])
        hwts[vt] = hwt

    # ==== final layernorm (feature-major) ====
    for sh in range(2):
        sum_ps = prow.tile([P, 512], F32, tag="prow")
        ssq_ps = prow.tile([P, 512], F32, tag="prow")
        for do in range(DSUB):
            nc.tensor.matmul(sum_ps[:1, :], ones_col_r[:], xfm[:, do, ts(sh, 512)],
                             start=(do == 0), stop=(do == DSUB - 1))
            sq = sqp.tile([P, 512], BF16, tag="sq")
            nc.scalar.activation(sq[:], xfm[:, do, ts(sh, 512)], AF.Square)
            nc.tensor.matmul(ssq_ps[:1, :], ones_col_b[:], sq[:],
                             start=(do == 0), stop=(do == DSUB - 1))
        mu = rows.tile([1, 512], F32, tag="srow")
        nc.vector.tensor_scalar_mul(mu[:], sum_ps[:1, :], 1.0 / D)
        var = rows.tile([1, 512], F32, tag="srow")
        nc.vector.tensor_scalar_mul(var[:], ssq_ps[:1, :], 1.0 / D)
        msq = rows.tile([1, 512], F32, tag="srow")
        nc.scalar.square(msq[:], mu[:])
        nc.vector.tensor_tensor(var[:], var[:], msq[:], ALU.subtract)
        nc.scalar.activation(var[:], var[:], AF.Sqrt, bias=eps5_t[:])
        nc.vector.reciprocal_approx_fast(out=msq[:], in_=var[:])
        nc.vector.tensor_scalar_mul(mu[:], mu[:], -1.0)
        nc.vector.tensor_tensor(var[:], mu[:], msq[:], ALU.mult)
        psb = pa.tile([P, 512], F32, tag="pa")
        nc.tensor.matmul(psb[:], ones_row_f[:], msq[:], start=True, stop=True)
        a_bc = ring.tile([P, 512], F32, tag="lnbc", bufs=2)
        nc.vector.tensor_copy(a_bc[:], psb[:])
        psb = pa.tile([P, 512], F32, tag="pa")
        nc.tensor.matmul(psb[:], ones_row_f[:], var[:], start=True, stop=True)
        b_bc = ring.tile([P, 512], F32, tag="lnbc", bufs=2)
        nc.vector.tensor_copy(b_bc[:], psb[:])
        for do in range(DSUB):
            t1 = scr.tile([P, 512], F32, tag="scr")
            nc.vector.tensor_tensor(t1[:], xfm[:, do, ts(sh, 512)], a_bc[:],
                                    ALU.mult)
            nc.vector.tensor_tensor(t1[:], t1[:], b_bc[:], ALU.add)
            nc.vector.tensor_scalar(t1[:], t1[:], lng_sb[:, do:do + 1],
                                    lnb_sb[:, do:do + 1], ALU.mult, ALU.add)
            ecopy(do, xfm[:, do, ts(sh, 512)], t1[:])

    # ==== vocab-shard head: logits_t[vt*128+vv, s] ====
    for vt in range(VTS):
        hwt = hwts.pop(vt)
        if vt + 2 < VTS:
            nxt = hwp.tile([P, DSUB, P], F32R, tag="hw")
            nc.sync.dma_start(nxt[:], hw_d[vt + 2])
            hwts[vt + 2] = nxt
        for sh in range(2):
            ps = pa.tile([P, 512], F32, tag="pa")
            for ko in range(DSUB):
                nc.tensor.matmul(ps[:], hwt[:, ko, :],
                                 xfm[:, ko, ts(sh, 512)],
                                 start=(ko == 0), stop=(ko == DSUB - 1))
            ot = outp.tile([P, 512], F32, tag="out")
            ecopy(sh, ot[:], ps[:])
            nc.sync.dma_start(out_d[ts(vt, P), ts(sh, 512)], ot[:])

    ctx.close()


def _round_f32r(x):
    m, e = np.frexp(x.astype(np.float64))
    return np.ldexp(np.round(m * 4096.0) / 4096.0, e).astype(np.float32)


_CACHE = {}


def _get_program():
    if "nc" not in _CACHE:
        _CACHE["nc"] = build_program()
    return _CACHE["nc"]


def make_in_maps(tokens, emb, Wq, Wk, Wv, Wb, Wo, rms_w, ln_g, ln_b, head_w):
    def arrange_w(w):  # [D, N] -> [128, DSUB, N] with (p, ko) striping of D
        return np.ascontiguousarray(
            _round_f32r(w).reshape(DSUB, P, -1).transpose(1, 0, 2))

    wq_h = np.stack([arrange_w(Wq[l]) for l in range(L)])
    wk_h = np.stack([arrange_w(Wk[l]) for l in range(L)])
    wv_h = np.stack([arrange_w(Wv[l]) for l in range(L)])
    wb_h = np.stack([arrange_w(np.repeat(Wb[l], 2, axis=1)) for l in range(L)])
    wo_h = np.stack([arrange_w(rms_w[l][:, None] * Wo[l]) for l in range(L)]).astype(ml_dtypes.bfloat16)
    emb_h = _round_f32r(emb)
    lng_h = np.ascontiguousarray(ln_g.reshape(DSUB, P).T)
    lnb_h = np.ascontiguousarray(ln_b.reshape(DSUB, P).T)

    in_maps = []
    for core in range(8):
        b, vs = core // 4, core % 4
        hw_pad = np.zeros((D, VSP), np.float32)
        hw_pad[:, :VS] = _round_f32r(head_w[:, ts(vs, VS)])
        hw_h = np.ascontiguousarray(
            hw_pad.reshape(DSUB, P, VTS, P).transpose(2, 1, 0, 3))
        tok_h = np.ascontiguousarray(
            tokens[b].astype(np.int32).reshape(NCH, P).T)
        in_maps.append({
            "tokens": tok_h, "emb": emb_h,
            "wq": wq_h, "wk": wk_h, "wv": wv_h, "wb": wb_h, "wo": wo_h,
            "lng": lng_h, "lnb": lnb_h, "hw": hw_h,
        })
    return in_maps


def assemble_output(results):
    out = np.empty((2, S, V), np.float32)
    for core in range(8):
        b, vs = core // 4, core % 4
        lt = results[core]["logits_t"]          # [VSP, S]
        out[b, :, ts(vs, VS)] = np.ascontiguousarray(lt[:VS]).T
    return out


def kernel(tokens, emb, Wq, Wk, Wv, Wb, Wo, rms_w, ln_g, ln_b, head_w):
    tokens = np.asarray(tokens)
    args = [np.asarray(a, np.float32) for a in
            (emb, Wq, Wk, Wv, Wb, Wo, rms_w, ln_g, ln_b, head_w)]
    nc = _get_program()
    in_maps = make_in_maps(tokens, *args)
    res = run_bass_kernel_spmd(nc, in_maps, core_ids=list(range(8)),
                               trace=bool(_CACHE.get("trace")))
    _CACHE["last_result"] = res
    return assemble_output(res.results)
